# Initial kernel scaffold
#
"""Multi-head attention (B=1, S=4096, dim=1024, 16 heads x 64) on 8 NeuronCores.

Sharding: tensor-parallel over heads. Core c computes heads {2c, 2c+1}:
  - Q/K/V projections for its 128 qkv-dims (x is replicated),
  - full attention for its 2 heads (flash-style, S^T layout, softmax
    denominator via an appended ones-column in the AV matmul),
  - its partial out-projection y_c = attn_out_c @ Wo[c*128:(c+1)*128, :].
Host unshards by summing the 8 partials and adding bo.

Matmuls run in float32r (TRN2 single-pass fp32, ~1e-3 rel err, 4x faster
than float32); softmax exp and normalization are fp32 on ScalarE/VectorE.
"""

import sys

sys.path.insert(0, "/opt/trn_rl_repo")

import numpy as np

import concourse.bass as bass
import concourse.mybir as mybir
import concourse.tile as tile
from concourse import bacc
from concourse.bass_utils import run_bass_kernel_spmd
from concourse.masks import make_identity

F32 = mybir.dt.float32
F32R = mybir.dt.float32r
AF = mybir.ActivationFunctionType

S = 4096          # sequence length
DIM = 1024        # model dim
NH = 16           # total heads
DK = 64           # head dim (= DV)
NCORES = 8
HPC = NH // NCORES          # heads per core (2)
DPC = HPC * DK              # qkv dims per core (128)
SCALE = DK ** -0.5

ST = S // 128               # 32 seq tiles of 128
JT = 8                      # x processed in 8 blocks of 512 rows
KT = DIM // 128             # 8 contraction tiles
QW = 1024                   # q-stripe width for attention
NT = S // QW                # 4 q-stripes


def build_bass():
    nc = bacc.Bacc(None)

    x = nc.declare_dram_parameter("x", [S, DIM], F32R, isOutput=False)
    wq = nc.declare_dram_parameter("wq", [DIM, DPC], F32R, isOutput=False)
    wk = nc.declare_dram_parameter("wk", [DIM, DPC], F32R, isOutput=False)
    wv = nc.declare_dram_parameter("wv", [DIM, DPC], F32R, isOutput=False)
    bq = nc.declare_dram_parameter("bq", [DPC, 1], F32, isOutput=False)
    bk = nc.declare_dram_parameter("bk", [DPC, 1], F32, isOutput=False)
    bv = nc.declare_dram_parameter("bv", [DPC, 1], F32, isOutput=False)
    wo = nc.declare_dram_parameter("wo", [DPC, DIM], F32R, isOutput=False)
    y = nc.declare_dram_parameter("y", [S, DIM], F32, isOutput=True)

    with tile.TileContext(nc) as tc:
        with (
            tc.tile_pool(name="const", bufs=1) as const,
            tc.tile_pool(name="persist", bufs=1) as persist,
            tc.tile_pool(name="work", bufs=2) as work,
            tc.tile_pool(name="pexp", bufs=2) as pexp,
            tc.tile_pool(name="dram", bufs=2, space="DRAM") as dram,
            tc.tile_pool(name="psum", bufs=1, space="PSUM") as psum,
        ):
            # ---- constants ----
            ident_f = const.tile([128, 128], F32)
            make_identity(nc, ident_f)
            ident = const.tile([128, 128], F32R)
            nc.vector.tensor_copy(ident[:], ident_f[:])
            ones_f = const.tile([128, 1], F32)
            nc.vector.memset(ones_f[:], 1.0)

            # ---- weights / biases ----
            wq_sb = const.tile([128, KT, DPC], F32R)
            wk_sb = const.tile([128, KT, DPC], F32R)
            wv_sb = const.tile([128, KT, DPC], F32R)
            nc.sync.dma_start(wq_sb[:], wq.rearrange("(kt p) d -> p kt d", p=128))
            nc.sync.dma_start(wk_sb[:], wk.rearrange("(kt p) d -> p kt d", p=128))
            nc.sync.dma_start(wv_sb[:], wv.rearrange("(kt p) d -> p kt d", p=128))
            wo_sb = const.tile([64, HPC, DIM], F32R)
            nc.sync.dma_start(wo_sb[:], wo.rearrange("(h p) m -> p h m", p=64))
            bq_sb = const.tile([DPC, 1], F32)
            bk_sb = const.tile([DPC, 1], F32)
            bv_sb = const.tile([DPC, 1], F32)
            nc.sync.dma_start(bq_sb[:], bq[:])
            nc.sync.dma_start(bk_sb[:], bk[:])
            nc.sync.dma_start(bv_sb[:], bv[:])

            # ---- persistent activations ----
            qT = persist.tile([DPC, S], F32R)        # Q^T: [d', s]
            kT = persist.tile([DPC, S], F32R)        # K^T: [d', s]
            # V natural per ks-tile: [ks, (V_h0 | 1 | V_h1 | 1)]
            v_nat = persist.tile([128, ST, 2 * (DK + 1)], F32R)
            uT = persist.tile([64, HPC, S], F32R)    # normalized attn out^T

            # ones columns of v_nat (softmax denominator trick)
            for st in range(ST):
                nc.vector.tensor_copy(v_nat[:, st, DK:DK + 1], ones_f[:])
                nc.vector.tensor_copy(v_nat[:, st, 2 * DK + 1:], ones_f[:])

            # ================= phase 1: x^T + projections =================
            for j in range(JT):
                xnat = work.tile([128, 4, DIM], F32R, tag="xn")
                nc.sync.dma_start(
                    xnat[:],
                    x[j * 512:(j + 1) * 512, :].rearrange("(a p) k -> p a k", p=128),
                )
                xt = work.tile([128, KT, 512], F32R, tag="xt")
                for kt in range(KT):
                    tp = psum.tile([128, 512], F32R, tag="tp")
                    for a in range(4):
                        nc.tensor.transpose(
                            tp[:, a * 128:(a + 1) * 128],
                            xnat[:, a, kt * 128:(kt + 1) * 128],
                            ident[:],
                        )
                    nc.vector.tensor_copy(xt[:, kt, :], tp[:])

                for w_sb, b_sb, dst in (
                    (wq_sb, bq_sb, qT),
                    (wk_sb, bk_sb, kT),
                    (wv_sb, bv_sb, None),
                ):
                    pp = psum.tile([128, 512], F32, tag="proj")
                    for kt in range(KT):
                        nc.tensor.matmul(
                            pp[:], w_sb[:, kt, :], xt[:, kt, :],
                            start=(kt == 0), stop=(kt == KT - 1),
                        )
                    if dst is not None:
                        nc.vector.tensor_scalar_add(
                            dst[:, j * 512:(j + 1) * 512], pp[:], b_sb[:]
                        )
                    else:
                        # V^T block -> bias -> transpose to natural layout
                        vt = work.tile([128, 512], F32R, tag="vt")
                        nc.vector.tensor_scalar_add(vt[:], pp[:], bv_sb[:])
                        tpv = psum.tile([128, 512], F32R, tag="tp")
                        for a in range(4):
                            nc.tensor.transpose(
                                tpv[:, a * 128:(a + 1) * 128],
                                vt[:, a * 128:(a + 1) * 128],
                                ident[:],
                            )
                        for a in range(4):
                            st = j * 4 + a
                            nc.vector.tensor_copy(
                                v_nat[:, st, 0:DK], tpv[:, a * 128:a * 128 + DK]
                            )
                            nc.vector.tensor_copy(
                                v_nat[:, st, DK + 1:2 * DK + 1],
                                tpv[:, a * 128 + DK:(a + 1) * 128],
                            )

            # ================= phase 2: attention =================
            for t in range(NT):
                for h in range(HPC):
                    hp = h * DK
                    u = psum.tile([DK + 1, QW], F32, tag="u")
                    for i in range(ST):
                        s_ps = psum.tile([128, QW], F32, tag="s")
                        lk = kT[hp:hp + DK, i * 128:(i + 1) * 128]
                        for q2 in range(QW // 512):
                            nc.tensor.matmul(
                                s_ps[:, q2 * 512:(q2 + 1) * 512],
                                lk,
                                qT[hp:hp + DK,
                                   t * QW + q2 * 512:t * QW + (q2 + 1) * 512],
                                start=True, stop=True,
                            )
                        p_sb = pexp.tile([128, QW], F32R, tag="p")
                        nc.scalar.activation(p_sb[:], s_ps[:], AF.Exp, scale=SCALE)
                        lv = v_nat[:, i, h * (DK + 1):(h + 1) * (DK + 1)]
                        for q2 in range(QW // 512):
                            nc.tensor.matmul(
                                u[:, q2 * 512:(q2 + 1) * 512],
                                lv,
                                p_sb[:, q2 * 512:(q2 + 1) * 512],
                                start=(i == 0), stop=(i == ST - 1),
                            )
                    # normalize: uT[:, h, t*QW:+QW] = u[0:64] / u[64]
                    rec = work.tile([1, QW], F32, tag="rec")
                    nc.vector.reciprocal(rec[:], u[DK:DK + 1, :])
                    rec_d = dram.tile([1, QW], F32)
                    nc.sync.dma_start(rec_d[:], rec[:])
                    rec_b = work.tile([64, QW], F32, tag="recb")
                    nc.gpsimd.dma_start(
                        rec_b[:],
                        bass.AP(tensor=rec_d.tensor, offset=rec_d.offset,
                                ap=[[0, 64], [1, QW]]),
                    )
                    nc.vector.tensor_mul(
                        uT[:, h, t * QW:(t + 1) * QW], u[0:DK, :], rec_b[:]
                    )

            # ================= phase 3: out-projection =================
            for q in range(ST):
                yp = psum.tile([128, DIM], F32, tag="y")
                for m in range(DIM // 512):
                    for h in range(HPC):
                        nc.tensor.matmul(
                            yp[:, m * 512:(m + 1) * 512],
                            uT[:, h, q * 128:(q + 1) * 128],
                            wo_sb[:, h, m * 512:(m + 1) * 512],
                            start=(h == 0), stop=(h == HPC - 1),
                        )
                ysb = work.tile([128, DIM], F32, tag="xn")
                nc.vector.tensor_copy(ysb[:], yp[:])
                nc.sync.dma_start(y[q * 128:(q + 1) * 128, :], ysb[:])

    nc.finalize()
    return nc


_NC_CACHE = None


def _get_nc():
    global _NC_CACHE
    if _NC_CACHE is None:
        _NC_CACHE = build_bass()
    return _NC_CACHE


def kernel(x, Wq, bq, Wk, bk, Wv, bv, Wo, bo, _want_results=False, **run_kwargs):
    x = np.ascontiguousarray(np.asarray(x, dtype=np.float32).reshape(S, DIM))
    Wq = np.asarray(Wq, dtype=np.float32)
    Wk = np.asarray(Wk, dtype=np.float32)
    Wv = np.asarray(Wv, dtype=np.float32)
    Wo = np.asarray(Wo, dtype=np.float32)
    bq = np.asarray(bq, dtype=np.float32)
    bk = np.asarray(bk, dtype=np.float32)
    bv = np.asarray(bv, dtype=np.float32)
    bo = np.asarray(bo, dtype=np.float32)

    nc = _get_nc()
    in_maps = []
    for c in range(NCORES):
        sl = slice(c * DPC, (c + 1) * DPC)
        in_maps.append({
            "x": x,
            "wq": np.ascontiguousarray(Wq[:, sl]),
            "wk": np.ascontiguousarray(Wk[:, sl]),
            "wv": np.ascontiguousarray(Wv[:, sl]),
            "bq": np.ascontiguousarray(bq[sl]).reshape(DPC, 1),
            "bk": np.ascontiguousarray(bk[sl]).reshape(DPC, 1),
            "bv": np.ascontiguousarray(bv[sl]).reshape(DPC, 1),
            "wo": np.ascontiguousarray(Wo[sl, :]),
        })
    res = run_bass_kernel_spmd(nc, in_maps, core_ids=list(range(NCORES)),
                               **run_kwargs)
    out = np.zeros((S, DIM), dtype=np.float64)
    for c in range(NCORES):
        out += res.results[c]["y"].astype(np.float64)
    out += bo.astype(np.float64)
    out = out.astype(np.float32).reshape(1, S, DIM)
    if _want_results:
        return out, res
    return out


# revision 6
# speedup vs baseline: 2.4206x; 2.4206x over previous
"""Multi-head attention (B=1, S=4096, dim=1024, 16 heads x 64) on 8 NeuronCores.

Sharding: tensor-parallel over heads. Core c computes heads {2c, 2c+1}:
  - Q/K/V projections for its 128 qkv-dims (x is replicated),
  - full attention for its 2 heads (flash-style, S^T layout, softmax
    denominator via an appended ones-column in the AV matmul),
  - its partial out-projection y_c = attn_out_c @ Wo[c*128:(c+1)*128, :].
Host unshards by summing the 8 partials and adding bo.

Matmul operands are fp16 (all intermediate values here are well within
fp16 range; rel err ~1e-3). Softmax runs in fp32 on ScalarE; all matmul
accumulation is fp32 in PSUM. x is transposed on the fly by the DMA xbar
transpose engine (2-byte dtype path).
"""

import sys

sys.path.insert(0, "/opt/trn_rl_repo")

import numpy as np

import concourse.bass as bass
import concourse.mybir as mybir
import concourse.tile as tile
from concourse import bacc
from concourse.bass_utils import run_bass_kernel_spmd

F32 = mybir.dt.float32
F16 = mybir.dt.float16
AF = mybir.ActivationFunctionType

S = 4096          # sequence length
DIM = 1024        # model dim
NH = 16           # total heads
DK = 64           # head dim (= DV)
NCORES = 8
HPC = NH // NCORES          # heads per core (2)
DPC = HPC * DK              # qkv dims per core (128)
SCALE = DK ** -0.5

ST = S // 128               # 32 seq tiles of 128
KT = DIM // 128             # 8 contraction tiles
QW = 512                    # q-stripe width for attention (per head)
NT = S // QW                # 8 q-stripes


def build_bass():
    nc = bacc.Bacc(None)

    x = nc.declare_dram_parameter("x", [S, DIM], F16, isOutput=False)
    wq = nc.declare_dram_parameter("wq", [DIM, DPC], F16, isOutput=False)
    wk = nc.declare_dram_parameter("wk", [DIM, DPC], F16, isOutput=False)
    wv = nc.declare_dram_parameter("wv", [DIM, DPC], F16, isOutput=False)
    bq = nc.declare_dram_parameter("bq", [DPC, 1], F32, isOutput=False)
    bk = nc.declare_dram_parameter("bk", [DPC, 1], F32, isOutput=False)
    bv = nc.declare_dram_parameter("bv", [DPC, 1], F32, isOutput=False)
    wo = nc.declare_dram_parameter("wo", [DPC, DIM], F16, isOutput=False)
    y = nc.declare_dram_parameter("y", [S, DIM], F32, isOutput=True)

    with tile.TileContext(nc) as tc:
        with (
            tc.tile_pool(name="const", bufs=1) as const,
            tc.tile_pool(name="persist", bufs=1) as persist,
            tc.tile_pool(name="work", bufs=2) as work,
            tc.tile_pool(name="pexp", bufs=3) as pexp,
            tc.tile_pool(name="dram", bufs=2, space="DRAM") as dram,
        ):
            # ---- constants / weights ----
            from concourse.masks import make_identity

            ident_f = const.tile([128, 128], F32)
            make_identity(nc, ident_f)
            ident = const.tile([128, 128], F16)
            nc.vector.tensor_copy(ident[:], ident_f[:])
            ones_f = const.tile([128, 1], F32)
            nc.vector.memset(ones_f[:], 1.0)

            wq_sb = const.tile([128, KT, DPC], F16)
            wk_sb = const.tile([128, KT, DPC], F16)
            wv_sb = const.tile([128, KT, DPC], F16)
            nc.sync.dma_start(wq_sb[:], wq.rearrange("(kt p) d -> p kt d", p=128))
            nc.sync.dma_start(wk_sb[:], wk.rearrange("(kt p) d -> p kt d", p=128))
            nc.sync.dma_start(wv_sb[:], wv.rearrange("(kt p) d -> p kt d", p=128))
            wo_sb = const.tile([DPC, DIM], F16)
            nc.sync.dma_start(wo_sb[:], wo[:])
            bq_sb = const.tile([DPC, 1], F32)
            bk_sb = const.tile([DPC, 1], F32)
            bv_sb = const.tile([DPC, 1], F32)
            nc.sync.dma_start(bq_sb[:], bq[:])
            nc.sync.dma_start(bk_sb[:], bk[:])
            nc.sync.dma_start(bv_sb[:], bv[:])

            # ---- persistent activations ----
            xT = persist.tile([128, KT, S], F16)      # x^T, via DMA xbar transpose
            qT = persist.tile([DPC, S], F16)          # Q^T: [d', s]
            kT = persist.tile([DPC, S], F16)          # K^T: [d', s]
            v_nat = persist.tile([128, ST, 2 * (DK + 1)], F16)
            uT = persist.tile([DPC, S], F16)          # normalized attn out^T

            for st in range(ST):
                nc.vector.tensor_copy(v_nat[:, st, DK:DK + 1], ones_f[:])
                nc.vector.tensor_copy(v_nat[:, st, 2 * DK + 1:], ones_f[:])

            # x^T: one xbar-transpose DMA per 128-column block of x
            for kt in range(KT):
                nc.sync.dma_start_transpose(
                    xT[:, kt, :], x[:, kt * 128:(kt + 1) * 128]
                )

            # ================= phase 1: projections =================
            with tc.tile_pool(name="psum1", bufs=1, space="PSUM") as psum1:
                for j in range(KT):  # 8 blocks of 512 seq positions
                    sl = slice(j * 512, (j + 1) * 512)
                    for w_sb, b_sb, dst, tag in (
                        (wq_sb, bq_sb, qT, "pq"),
                        (wk_sb, bk_sb, kT, "pk"),
                        (wv_sb, bv_sb, None, "pv"),
                    ):
                        pp = psum1.tile([128, 512], F32, tag=tag)
                        for kt in range(KT):
                            nc.tensor.matmul(
                                pp[:], w_sb[:, kt, :], xT[:, kt, sl],
                                start=(kt == 0), stop=(kt == KT - 1),
                            )
                        if dst is not None:
                            nc.vector.tensor_scalar_add(pp_out(dst, sl), pp[:], b_sb[:])
                        else:
                            vt = work.tile([128, 512], F16, tag="vt")
                            nc.vector.tensor_scalar_add(vt[:], pp[:], b_sb[:])
                            tpv = psum1.tile([128, 512], F16, tag="tp")
                            for a in range(4):
                                nc.tensor.transpose(
                                    tpv[:, a * 128:(a + 1) * 128],
                                    vt[:, a * 128:(a + 1) * 128],
                                    ident[:],
                                )
                            for a in range(4):
                                st = j * 4 + a
                                nc.vector.tensor_copy(
                                    v_nat[:, st, 0:DK],
                                    tpv[:, a * 128:a * 128 + DK],
                                )
                                nc.vector.tensor_copy(
                                    v_nat[:, st, DK + 1:2 * DK + 1],
                                    tpv[:, a * 128 + DK:(a + 1) * 128],
                                )

            # ================= phase 2: attention =================
            # Per stripe t (512 q) both heads run together: the two K=64
            # score matmuls occupy disjoint PE quadrant rows (h0 at array
            # rows 0-63, h1 at 64-127) and execute concurrently; one
            # [128, 1024] exp covers both heads' score banks.
            with tc.tile_pool(name="psum2", bufs=2, space="PSUM") as psum2:
                for t in range(NT):
                    qsl = slice(t * QW, (t + 1) * QW)
                    u0 = psum2.tile([DK + 1, QW], F32, tag="u0")
                    u1 = psum2.tile([DK + 1, QW], F32, tag="u1")
                    for i in range(ST):
                        s_ps = psum2.tile([128, 2 * QW], F32, tag="s")
                        for h, u in ((0, u0), (1, u1)):
                            hp = h * DK
                            nc.tensor.matmul(
                                s_ps[:, h * QW:(h + 1) * QW],
                                kT[hp:hp + DK, i * 128:(i + 1) * 128],
                                qT[hp:hp + DK, qsl],
                                start=True, stop=True,
                            )
                        p_sb = pexp.tile([128, 2 * QW], F16, tag="p")
                        nc.scalar.activation(p_sb[:], s_ps[:], AF.Exp,
                                             scale=SCALE)
                        for h, u in ((0, u0), (1, u1)):
                            nc.tensor.matmul(
                                u[:],
                                v_nat[:, i, h * (DK + 1):(h + 1) * (DK + 1)],
                                p_sb[:, h * QW:(h + 1) * QW],
                                start=(i == 0), stop=(i == ST - 1),
                            )
                    # normalize: uT[h*64:(h+1)*64, qsl] = u[0:64] / u[64]
                    for h, u in ((0, u0), (1, u1)):
                        rec = work.tile([1, QW], F32, tag="rec")
                        nc.vector.reciprocal(rec[:], u[DK:DK + 1, :])
                        rec_d = dram.tile([1, QW], F32)
                        nc.sync.dma_start(rec_d[:], rec[:])
                        rec_b = work.tile([64, QW], F32, tag="recb")
                        nc.gpsimd.dma_start(
                            rec_b[:],
                            bass.AP(tensor=rec_d.tensor, offset=rec_d.offset,
                                    ap=[[0, 64], [1, QW]]),
                        )
                        if h == 0:
                            nc.vector.tensor_mul(uT[0:DK, qsl], u[0:DK, :],
                                                 rec_b[:])
                        else:
                            # DVE lanes can't shift partitions: go via SBUF
                            # then DMA down to partitions 64-127.
                            ush = work.tile([DK, QW], F16, tag="ush")
                            nc.vector.tensor_mul(ush[:], u[0:DK, :], rec_b[:])
                            nc.gpsimd.dma_start(uT[DK:2 * DK, qsl], ush[:])

            # ================= phase 3: out-projection =================
            with tc.tile_pool(name="psum3", bufs=2, space="PSUM") as psum3:
                for q in range(ST):
                    yp = psum3.tile([128, DIM], F32, tag="y")
                    for m in range(DIM // 512):
                        nc.tensor.matmul(
                            yp[:, m * 512:(m + 1) * 512],
                            uT[:, q * 128:(q + 1) * 128],
                            wo_sb[:, m * 512:(m + 1) * 512],
                            start=True, stop=True,
                        )
                    ysb = work.tile([128, DIM], F32, tag="ysb")
                    nc.vector.tensor_copy(ysb[:], yp[:])
                    nc.sync.dma_start(y[q * 128:(q + 1) * 128, :], ysb[:])

    nc.finalize()
    return nc


def pp_out(dst, sl):
    return dst[:, sl]


_NC_CACHE = None


def _get_nc():
    global _NC_CACHE
    if _NC_CACHE is None:
        _NC_CACHE = build_bass()
    return _NC_CACHE


def kernel(x, Wq, bq, Wk, bk, Wv, bv, Wo, bo, _want_results=False, **run_kwargs):
    x = np.ascontiguousarray(
        np.asarray(x, dtype=np.float32).reshape(S, DIM)).astype(np.float16)
    Wq = np.asarray(Wq, dtype=np.float32).astype(np.float16)
    Wk = np.asarray(Wk, dtype=np.float32).astype(np.float16)
    Wv = np.asarray(Wv, dtype=np.float32).astype(np.float16)
    Wo = np.asarray(Wo, dtype=np.float32).astype(np.float16)
    bq = np.asarray(bq, dtype=np.float32)
    bk = np.asarray(bk, dtype=np.float32)
    bv = np.asarray(bv, dtype=np.float32)
    bo = np.asarray(bo, dtype=np.float32)

    nc = _get_nc()
    in_maps = []
    for c in range(NCORES):
        sl = slice(c * DPC, (c + 1) * DPC)
        in_maps.append({
            "x": x,
            "wq": np.ascontiguousarray(Wq[:, sl]),
            "wk": np.ascontiguousarray(Wk[:, sl]),
            "wv": np.ascontiguousarray(Wv[:, sl]),
            "bq": np.ascontiguousarray(bq[sl]).reshape(DPC, 1),
            "bk": np.ascontiguousarray(bk[sl]).reshape(DPC, 1),
            "bv": np.ascontiguousarray(bv[sl]).reshape(DPC, 1),
            "wo": np.ascontiguousarray(Wo[sl, :]),
        })
    res = run_bass_kernel_spmd(nc, in_maps, core_ids=list(range(NCORES)),
                               **run_kwargs)
    out = np.zeros((S, DIM), dtype=np.float64)
    for c in range(NCORES):
        out += res.results[c]["y"].astype(np.float64)
    out += bo.astype(np.float64)
    out = out.astype(np.float32).reshape(1, S, DIM)
    if _want_results:
        return out, res
    return out
